# revision 1
# baseline (speedup 1.0000x reference)
"""CondConv2D Trainium2 kernel.

Problem (hardcoded shapes): B=16, C_in=64, H=W=256, E=4, C_out=64, 3x3 conv,
stride=1, dilation=1, padding=1.

Sharding: data-parallel over batch. 8 cores x 2 images each. Expert weights
and routing fc params replicated (host pre-transposed for layout only).

Per-core kernel (single pass over HBM):
  - Each image resident in SBUF as 10 tiles [128, 13, 258]: partitions 0-63
    hold channels of the TOP half rows (-1..128, incl. zero pad row), 64-127
    the BOTTOM half rows (127..256). 258 = 256 + 2 zero pad cols.
  - Routing: per-tile reduce (DVE) -> pooled sums; fc via elementwise mul +
    ones-matmul (contracts the 128 partitions and broadcasts the logits to
    all partitions); + bias; sigmoid (ACT) -> r [128, 4].
  - Mixed kernels: W_mix[c_in, tap*64+c_out] = sum_e r_e * W[e,...] on DVE.
    Both partition halves hold identical copies.
  - Conv: per output row-pair, 9 taps as fp32r matmuls (K=64 c_in,
    M=64 c_out, N=512 = 2 rows x 256 px) accumulated in PSUM. Shifted-view
    rhs APs give the im2col for free. tile_position packs top-half and
    bottom-half matmuls into disjoint PE array quadrants so they run
    concurrently (NPX=2 adds a second pixel-tile pair for 4-way packing).
"""
import sys

if "/opt/trn_rl_repo" not in sys.path:
    sys.path.insert(0, "/opt/trn_rl_repo")

import numpy as np

import concourse.bacc as bacc
import concourse.mybir as mybir
import concourse.tile as tile
from concourse.bass_utils import run_bass_kernel_spmd

F32 = mybir.dt.float32
F32R = mybir.dt.float32r
BF16 = mybir.dt.bfloat16
AF = mybir.ActivationFunctionType
ALU = mybir.AluOpType

N_CORES = 8
IMGS_PER_CORE = 2
C_IN = 64
C_OUT = 64
H = 256
W = 256
E = 4
NTAP = 9
ROWS_PER_TILE = 13
N_TILES = 10          # 130 lines per half
HALF = 128            # output rows per half

NPX = 2               # pixel-row-pairs processed concurrently (1 or 2)
IMG_BUFS = 12         # resident tile slots (10 = one full image)
PSUM_BUFS = 6
STAGE_ROWS = 16       # output rows per half per staging tile


def build_nc(npx=NPX):
    nc = bacc.Bacc("TRN2", target_bir_lowering=False, debug=False,
                   num_devices=N_CORES)
    x = nc.dram_tensor("x", [IMGS_PER_CORE, C_IN, H, W], F32,
                       kind="ExternalInput")
    wt = nc.dram_tensor("wt", [128, E * NTAP * C_OUT], F32,
                        kind="ExternalInput")
    fcw = nc.dram_tensor("fcw", [128, E], F32, kind="ExternalInput")
    fcb = nc.dram_tensor("fcb", [128, E], F32, kind="ExternalInput")
    ones = nc.dram_tensor("ones", [128, 128], F32, kind="ExternalInput")
    y = nc.dram_tensor("y", [IMGS_PER_CORE, C_OUT, H, W], F32,
                       kind="ExternalOutput")

    with tile.TileContext(nc) as tc:
        with (
            tc.tile_pool(name="consts", bufs=1) as consts,
            tc.tile_pool(name="img", bufs=IMG_BUFS) as img_pool,
            tc.tile_pool(name="small", bufs=2) as small,
            tc.tile_pool(name="mix", bufs=2) as mix_pool,
            tc.tile_pool(name="stage", bufs=2) as stage_pool,
            tc.tile_pool(name="psum", bufs=PSUM_BUFS, space="PSUM") as psum_pool,
        ):
            wtt = consts.tile([128, E * NTAP * C_OUT], BF16)
            fcwt = consts.tile([128, E], F32)
            fcbt = consts.tile([128, E], F32)
            onest = consts.tile([128, 128], F32)
            nc.gpsimd.dma_start(wtt[:], wt[:])
            nc.sync.dma_start(fcwt[:], fcw[:])
            nc.sync.dma_start(fcbt[:], fcb[:])
            nc.sync.dma_start(onest[:], ones[:])

            for i in range(IMGS_PER_CORE):
                # ---- load resident tiles + per-tile pooling reduces ----
                xt = []
                partial = small.tile([128, N_TILES], F32)
                for t in range(N_TILES):
                    # unpadded, contiguous per-partition: the f32->bf16 cast
                    # DMA (SWDGE) then needs only one descriptor per
                    # partition. Edge-column handling moved into the conv
                    # matmuls (shifted psum windows).
                    xtile = img_pool.tile([128, ROWS_PER_TILE, W], BF16)
                    xt.append(xtile)
                    # top half: global rows 13t-1 .. 13t+11
                    g0 = 13 * t - 1
                    if t == 0:
                        # zero row -1; overlap line 1 (DMA rewrites it) so
                        # Tile orders memset before the DMA.
                        nc.vector.memset(xtile[0:64, 0:2, :], 0.0)
                        nc.gpsimd.dma_start(xtile[0:64, 1:13, :],
                                            x[i, :, 0:12, :])
                    else:
                        nc.gpsimd.dma_start(xtile[0:64, :, :],
                                            x[i, :, g0:g0 + 13, :])
                    # bottom half: global rows 127+13t .. 139+13t
                    b0 = 127 + 13 * t
                    if t == N_TILES - 1:
                        nc.vector.memset(xtile[64:128, 11:13, :], 0.0)
                        nc.gpsimd.dma_start(xtile[64:128, 0:12, :],
                                            x[i, :, b0:b0 + 12, :])
                    else:
                        nc.gpsimd.dma_start(xtile[64:128, :, :],
                                            x[i, :, b0:b0 + 13, :])
                    # pooling partial sums (pad cols/rows are zero).
                    # bottom tile 0 lines 0,1 = rows 127,128 already counted
                    # in the top half -> exclude.
                    if t == 0:
                        nc.vector.reduce_sum(partial[0:64, 0:1],
                                             xtile[0:64],
                                             axis=mybir.AxisListType.XY)
                        nc.vector.reduce_sum(partial[64:128, 0:1],
                                             xtile[64:128, 2:13, :],
                                             axis=mybir.AxisListType.XY)
                    else:
                        nc.vector.reduce_sum(partial[:, t:t + 1],
                                             xtile[:],
                                             axis=mybir.AxisListType.XY)

                # ---- routing ----
                pooled = small.tile([128, 1], F32)
                nc.vector.reduce_sum(pooled[:], partial[:],
                                     axis=mybir.AxisListType.X)
                tmp4 = small.tile([128, E], F32)
                nc.vector.tensor_scalar(tmp4[:], fcwt[:], pooled[:, 0:1],
                                        1.0 / float(H * W),
                                        op0=ALU.mult, op1=ALU.mult)
                ps4 = psum_pool.tile([128, E], F32, bufs=1)
                nc.tensor.matmul(ps4[:], onest[:], tmp4[:], start=True,
                                 stop=True)
                logits = small.tile([128, E], F32)
                nc.vector.tensor_tensor(logits[:], ps4[:], fcbt[:], op=ALU.add)
                rt = small.tile([128, E], F32)
                nc.scalar.activation(rt[:], logits[:], AF.Sigmoid)

                # ---- mix expert kernels ----
                wmix = mix_pool.tile([128, NTAP * C_OUT], BF16)
                wtmp = mix_pool.tile([128, NTAP * C_OUT], BF16)
                S = NTAP * C_OUT
                nc.vector.tensor_scalar_mul(wmix[:], wtt[:, 0:S], rt[:, 0:1])
                for e in range(1, E):
                    nc.vector.tensor_scalar_mul(wtmp[:], wtt[:, e * S:(e + 1) * S],
                                                rt[:, e:e + 1])
                    nc.vector.tensor_tensor(wmix[:], wmix[:], wtmp[:], op=ALU.add)

                # ---- conv ----
                n_pairs = HALF // 2                     # 64 row-pairs per half
                groups_per_stage = STAGE_ROWS // (2 * npx)
                stage = None
                for g in range(n_pairs // npx):
                    if g % groups_per_stage == 0:
                        stage = stage_pool.tile([128, STAGE_ROWS, W], F32)
                    ps_tiles = [psum_pool.tile([128, 2, W], F32, name="ps",
                                               tag="ps")
                                for _ in range(npx)]
                    # Per-pair tap order constraints:
                    #  - first tap must write the full psum region with
                    #    start=True -> must be a center tap (kw=1) of an
                    #    unsplit kh (not straddling a tile boundary).
                    #  - a split kh must not be first or last.
                    tap_orders = []
                    for px in range(npx):
                        pair = npx * g + px
                        split_kh = next((kh for kh in range(3)
                                         if (2 * pair + kh) % ROWS_PER_TILE
                                         == ROWS_PER_TILE - 1), None)
                        if split_kh is None:
                            seq = [0, 1, 2]
                        else:
                            others = [kh for kh in range(3) if kh != split_kh]
                            seq = [others[0], split_kh, others[1]]
                        order = [seq[0] * 3 + 1, seq[0] * 3 + 0, seq[0] * 3 + 2]
                        for kh in seq[1:]:
                            order += [kh * 3 + 0, kh * 3 + 1, kh * 3 + 2]
                        tap_orders.append(order)
                    for r in range(NTAP):
                        st = r == 0
                        sp = r == NTAP - 1
                        for px in range(npx):
                            pair = npx * g + px
                            tap = tap_orders[px][r]
                            kh, kw = divmod(tap, 3)
                            L = 2 * pair + kh
                            ps = ps_tiles[px]
                            # x col window <- out col window (dx = kw-1)
                            if kw == 0:
                                xs, xe, os0, oe = 0, W - 1, 1, W
                            elif kw == 1:
                                xs, xe, os0, oe = 0, W, 0, W
                            else:
                                xs, xe, os0, oe = 1, W, 0, W - 1
                            t, m = divmod(L, ROWS_PER_TILE)
                            unsplit = m <= ROWS_PER_TILE - 2
                            for half in range(2):
                                hs = slice(0, 64) if half == 0 else slice(64, 128)
                                lhsT = wmix[hs, tap * 64:(tap + 1) * 64]
                                if px == 0:
                                    tp = (0, 0) if half == 0 else (64, 64)
                                    osl = hs
                                else:
                                    tp = (0, 64) if half == 0 else (64, 0)
                                    osl = slice(64, 128) if half == 0 else slice(0, 64)
                                if kw == 1 and unsplit:
                                    # single N=512 matmul over both rows
                                    rhs = xt[t][hs, m:m + 2, :]
                                    rhs = rhs.rearrange("p a b -> p (a b)")
                                    out = ps[osl].rearrange("p a b -> p (a b)")
                                    nc.tensor.matmul(out, lhsT, rhs,
                                                     start=st, stop=sp,
                                                     tile_position=tp,
                                                     skip_group_check=True)
                                else:
                                    # one matmul per output row (2D APs)
                                    for j in range(2):
                                        tj, mj = divmod(L + j, ROWS_PER_TILE)
                                        rhs = xt[tj][hs, mj, xs:xe]
                                        nc.tensor.matmul(ps[osl, j, os0:oe],
                                                         lhsT, rhs,
                                                         start=st, stop=sp,
                                                         tile_position=tp,
                                                         skip_group_check=True)
                    # drain psum -> staging
                    r0 = (g % groups_per_stage) * 2 * npx
                    for px in range(npx):
                        dst = stage[:, r0 + 2 * px:r0 + 2 * px + 2, :]
                        src = ps_tiles[px][:]
                        if (g + px) % 2 == 0:
                            nc.vector.tensor_copy(dst, src)
                        else:
                            nc.scalar.copy(dst, src)
                    # staging full -> DMA out
                    if (g + 1) % groups_per_stage == 0:
                        mrow = (g // groups_per_stage) * STAGE_ROWS
                        if npx == 1:
                            nc.scalar.dma_start(y[i, :, mrow:mrow + STAGE_ROWS, :],
                                                stage[0:64])
                            nc.scalar.dma_start(
                                y[i, :, HALF + mrow:HALF + mrow + STAGE_ROWS, :],
                                stage[64:128])
                        else:
                            # stage blocks alternate psumA/psumB:
                            #  [0:64]   A: top (4j,4j+1)   B: bottom (4j+2,4j+3)
                            #  [64:128] A: bottom (4j,4j+1) B: top (4j+2,4j+3)
                            nj = STAGE_ROWS // 4
                            sv = stage.rearrange("p (j b r) w -> p j b r w",
                                                 j=nj, b=2, r=2)
                            ys = y[i].rearrange("c (blk four) w -> c blk four w",
                                                four=4)
                            # top rows from A blocks: rows mrow+4j+{0,1}
                            nc.scalar.dma_start(
                                ys[:, mrow // 4:mrow // 4 + nj, 0:2, :],
                                sv[0:64, :, 0, :, :])
                            # bottom rows from B blocks: rows 128+mrow+4j+{2,3}
                            nc.scalar.dma_start(
                                ys[:, (HALF + mrow) // 4:(HALF + mrow) // 4 + nj, 2:4, :],
                                sv[0:64, :, 1, :, :])
                            # bottom rows from A blocks: rows 128+mrow+4j+{0,1}
                            nc.scalar.dma_start(
                                ys[:, (HALF + mrow) // 4:(HALF + mrow) // 4 + nj, 0:2, :],
                                sv[64:128, :, 0, :, :])
                            # top rows from B blocks: rows mrow+4j+{2,3}
                            nc.scalar.dma_start(
                                ys[:, mrow // 4:mrow // 4 + nj, 2:4, :],
                                sv[64:128, :, 1, :, :])
    nc.compile()
    return nc


_NC_CACHE = {}


def _get_nc(npx=NPX):
    if npx not in _NC_CACHE:
        _NC_CACHE[npx] = build_nc(npx)
    return _NC_CACHE[npx]


def _prep_shared(weight, fc_w, fc_b):
    # [E, O, I, KH, KW] -> [I, E, KH, KW, O] -> [64, E*9*64], dup halves
    wt = np.ascontiguousarray(weight.transpose(2, 0, 3, 4, 1)).reshape(
        C_IN, E * NTAP * C_OUT)
    wt = np.concatenate([wt, wt], axis=0).astype(np.float32)
    fcw = np.concatenate([fc_w.T, fc_w.T], axis=0).astype(np.float32)
    fcb = np.tile(fc_b.reshape(1, E), (128, 1)).astype(np.float32)
    ones = np.ones((128, 128), np.float32)
    return wt, fcw, fcb, ones


def kernel(inputs, weight, fc_w, fc_b, stride=1, dilation=1, padding=1,
           _trace=False, _npx=NPX):
    assert int(stride) == 1 and int(dilation) == 1 and int(padding) == 1
    inputs = np.asarray(inputs, dtype=np.float32)
    B = inputs.shape[0]
    assert B == N_CORES * IMGS_PER_CORE
    wt, fcw, fcb, ones = _prep_shared(np.asarray(weight), np.asarray(fc_w),
                                      np.asarray(fc_b))
    nc = _get_nc(_npx)
    in_maps = []
    for c in range(N_CORES):
        in_maps.append({
            "x": np.ascontiguousarray(inputs[2 * c:2 * c + 2]),
            "wt": wt, "fcw": fcw, "fcb": fcb, "ones": ones,
        })
    res = run_bass_kernel_spmd(nc, in_maps, core_ids=list(range(N_CORES)),
                               trace=_trace)
    out = np.concatenate([res.results[c]["y"] for c in range(N_CORES)], axis=0)
    if _trace:
        return out, res
    return out

